# revision 20
# baseline (speedup 1.0000x reference)
"""Trainium2 Bass kernel for nn_CrossAttention_36309653521078.

Math notes:
  - seq_len == 1 => softmax over a single key is identically 1.0, so
    attn == V exactly; Q/K/score computation is dead code.
  - Wo folds into Wv host-side (exact, batch-independent):
        h_attn = (h_s @ Wv_flat + bv) @ Wo + bo
               = h_s @ (Wv_flat @ Wo) + (bv @ Wo + bo)
    so the device chain per row b is
        x1   = h_s @ W' + (h_g + b')          [B, 512]
        ha   = LN(x1) * g1 + b1_ln
        mlp  = gelu(ha @ W1 + b1) @ W2 + b2
        out  = LN(mlp + ha) * g2 + b2_ln
  - Device works feature-major (activations stored transposed [D, B]):
    every matmul takes W[din, dout] as the stationary operand and the
    activation [din, b] as the moving operand. The host pre-transposes
    h_s / h_g once and transposes the output back.
  - All matmul operands are bf16 (1 cyc/row, FWL-eligible weight loads);
    PSUM accumulation and LN statistics stay fp32.
  - Software pipelining: batch-tile t+1's h_s @ W' matmuls are emitted
    between tile t's LN1-stat matmuls and its MLP, so the PE stays busy
    through the LN stat chains. W' chunk loads interleave with the first
    tile's h_s streaming so the PE starts within a few us of launch.
Sharding: pure data parallelism over the batch dim across 8 cores.
"""

import numpy as np

import concourse.bass as bass
import concourse.mybir as mybir
import concourse.tile as tile
from concourse.bass_utils import run_bass_kernel_spmd

F32 = mybir.dt.float32
MM_DT = mybir.dt.bfloat16

N_CORES = 8
B = 16384
G_DIM = 512
S_DIM = 3072
HID = 512
H2 = 1024
BL = B // N_CORES      # rows per core
NB = 512               # batch-tile (moving free dim; psum bank max)
NBT = BL // NB         # batch tiles per core
KSUB = 8               # h_s k-subtiles (of 128) per streamed DMA chunk
EPS = 1e-5

SK = S_DIM // 128      # 24
KO = HID // 128        # 4
MH = H2 // 128         # 8
NKG = SK // KSUB       # 6

# consts tile column layout (each entry is [128, n] chunks of a vector)
_C_B1 = 0              # b1               [1024] -> cols 0:8
_C_B2 = 8              # b2               [512]  -> cols 8:12
_C_L1G = 12            # ln1_g            cols 12:16
_C_L1B = 16            # ln1_b            cols 16:20
_C_L2G = 20            # ln2_g            cols 20:24
_C_L2B = 24            # ln2_b            cols 24:28
_C_N = 28


def _split_multi_waits(nc):
    """The walrus build here rejects >1 sync-wait on several instruction
    codegen structs (Drain/CTRL, fused-LDW matmul). Hoist extra waits onto
    single-wait NOPs inserted just before the owning instruction."""
    for blk in nc.m.functions[0].blocks:
        insts = list(blk.instructions)
        out, changed, k = [], False, 0
        for inst in insts:
            si = inst.sync_info
            waits = list(si.on_wait) if si and si.on_wait else []
            if len(waits) > 1:
                for w in waits[:-1]:
                    out.append(mybir.InstNoOp(
                        name=f"wsplit-{blk.name}-{k}",
                        engine=inst.engine,
                        bass_nofuse=True,
                        sync_info=mybir.SyncInfo(on_wait=[w], on_update=[]),
                    ))
                    k += 1
                si.on_wait = [waits[-1]]
                changed = True
            out.append(inst)
        if changed:
            blk.instructions = out


def build_nc(reps: int = 1, split_waits: bool = True, timing: bool = False):
    """reps>1 repeats the whole per-core body (for differential timing).
    timing=True shrinks the DRAM activations (one batch-tile, re-read for
    every batch-tile) and adds a reps-sized marker output so that timing
    variants can't collide in any executable cache. split_waits must be
    True for HW (walrus); CoreSim needs False."""
    nc = bass.Bass("TRN2", target_bir_lowering=False, debug=False)

    bl = NB if timing else BL
    hs_rows = KSUB * 128 if timing else S_DIM
    wv_rows = KSUB * 128 if timing else S_DIM
    hsT = nc.dram_tensor("hsT", [hs_rows, bl], MM_DT, kind="ExternalInput").ap()
    hgT = nc.dram_tensor("hgT", [HID, bl], MM_DT, kind="ExternalInput").ap()
    wv = nc.dram_tensor("wv", [wv_rows, HID], MM_DT, kind="ExternalInput").ap()
    w1 = nc.dram_tensor("w1", [HID, H2], MM_DT, kind="ExternalInput").ap()
    w2 = nc.dram_tensor("w2", [H2, HID], MM_DT, kind="ExternalInput").ap()
    cst = nc.dram_tensor("cst", [128, _C_N], F32, kind="ExternalInput").ap()
    outT = nc.dram_tensor("outT", [HID, bl], MM_DT, kind="ExternalOutput").ap()
    mark = None
    if timing:
        mark = nc.dram_tensor("mark", [1, 8 * reps], F32,
                              kind="ExternalOutput").ap()

    n_kg = hs_rows // (KSUB * 128)          # 1 in timing mode, 6 real
    n_wv_kg = wv_rows // (KSUB * 128)       # 1 in timing mode, 6 real
    hsT_t = hsT.rearrange("(kg kk p) b -> kg p kk b", kk=KSUB, p=128)
    wv_t = wv.rearrange("(kg kk p) n -> kg p kk n", kk=KSUB, p=128)
    hgT_t = hgT.rearrange("(c p) b -> p c b", p=128)
    outT_t = outT.rearrange("(c p) b -> p c b", p=128)

    with tile.TileContext(nc) as tc:
        with (
            nc.allow_low_precision(
                reason="bf16 matmul operands by design; fp32 accumulate"),
            tc.tile_pool(name="weights", bufs=1) as wpool,
            tc.tile_pool(name="hs", bufs=6) as hs_pool,
            tc.tile_pool(name="hg", bufs=2) as hg_pool,
            tc.tile_pool(name="x1", bufs=2) as x1_pool,
            tc.tile_pool(name="g", bufs=10) as g_pool,
            tc.tile_pool(name="xsq", bufs=8) as xsq_pool,
            tc.tile_pool(name="stat", bufs=4) as stat_pool,
            tc.tile_pool(name="out", bufs=2) as out_pool,
            tc.tile_pool(name="psv", bufs=2, space="PSUM") as psv,
            tc.tile_pool(name="pstat", bufs=2, space="PSUM") as pstat,
            tc.tile_pool(name="pmlp", bufs=4, space="PSUM") as pmlp,
        ):
            # ---- early consts + ones (tiny) ----
            consts = wpool.tile([128, _C_N], F32)
            nc.sync.dma_start(out=consts, in_=cst)
            ones_f = wpool.tile([128, 128], F32)
            nc.vector.memset(ones_f, 1.0)
            ones128 = wpool.tile([128, 128], MM_DT)
            nc.scalar.activation(ones128, ones_f,
                                 mybir.ActivationFunctionType.Copy)
            eps_col = wpool.tile([128, 1], F32)
            nc.vector.memset(eps_col, EPS)
            mark_sb = None
            if timing:
                mark_sb = wpool.tile([1, 8], F32)
                nc.vector.memset(mark_sb, 1.0)

            wv_sb = wpool.tile([128, SK, HID], MM_DT)
            w1_sb = wpool.tile([128, KO, H2], MM_DT)
            w2_sb = wpool.tile([128, MH, HID], MM_DT)

            inv_n = 1.0 / HID

            def prefetch_hs(rep, t, first=False):
                """Issue all of a tile's h_s chunk DMAs (and interleave the
                W' chunk loads on the very first tile) well before the MMs
                need them."""
                bsl = slice(0, NB) if timing else slice(t * NB, (t + 1) * NB)
                hs_tiles = []
                H = KSUB // 2
                for kg in range(NKG):
                    hs_t = hs_pool.tile([128, KSUB, NB], MM_DT, name="hs_t")
                    if first:
                        # half-chunk interleave so the first MMs wait for
                        # ~1MB of DMA instead of 2MB at kernel start
                        for h in range(2):
                            hsl = slice(h * H, (h + 1) * H)
                            nc.sync.dma_start(
                                out=wv_sb[:, kg * KSUB + h * H:
                                          kg * KSUB + (h + 1) * H, :],
                                in_=wv_t[kg % n_wv_kg, :, hsl])
                            nc.sync.dma_start(
                                out=hs_t[:, hsl, :],
                                in_=hsT_t[kg % n_kg, :, hsl, bsl])
                    else:
                        nc.sync.dma_start(out=hs_t,
                                          in_=hsT_t[kg % n_kg, :, :, bsl])
                    hs_tiles.append(hs_t)
                hg_t = hg_pool.tile([128, KO, NB], MM_DT, name="hg_t")
                nc.scalar.dma_start(out=hg_t, in_=hgT_t[:, :, bsl])
                return hs_tiles, hg_t

            def emit_vprime(rep, t, hs_tiles, hg_t):
                """x1 = h_s @ W' + (h_g + b') for one batch-tile. m is the
                OUTER loop: 24 consecutive MMs accumulate into one PSUM
                bank, and the bank is evacuated (residual add -> SBUF x1)
                right after its stop-MM, so only 2 PSUM banks are needed
                and the adds pipeline with the next m's matmuls."""
                x1 = x1_pool.tile([128, KO, NB], MM_DT, tag="x1",
                                  name=f"x1_{rep}_{t}")
                for m in range(KO):
                    pv = psv.tile([128, NB], F32, tag="psv",
                                  name=f"psv{rep}_{t}_{m}")
                    for kg in range(NKG):
                        for kk in range(KSUB):
                            k = kg * KSUB + kk
                            nc.tensor.matmul(
                                pv,
                                wv_sb[:, k, m * 128:(m + 1) * 128],
                                hs_tiles[kg][:, kk, :],
                                start=(k == 0), stop=(k == SK - 1),
                            )
                    nc.vector.tensor_add(x1[:, m, :], pv, hg_t[:, m, :])
                return x1

            def emit_ln_stats(x, pfx):
                """sum / sumsq over the feature (partition) axis via
                all-ones stationary matmuls; results broadcast-rows.
                All xsq DVE ops are emitted first so the DVE has a head
                start before the PE reaches the sumsq matmuls."""
                xsqs = []
                for j in range(KO):
                    xsq = xsq_pool.tile([128, NB], MM_DT, tag="xsq",
                                        name=f"{pfx}xsq{j}")
                    nc.vector.tensor_mul(xsq, x[:, j, :], x[:, j, :])
                    xsqs.append(xsq)
                sumB = pstat.tile([128, NB], F32, tag="pstat", name=f"{pfx}s")
                for j in range(KO):
                    nc.tensor.matmul(sumB, ones128, x[:, j, :],
                                     start=(j == 0), stop=(j == KO - 1))
                sqB = pstat.tile([128, NB], F32, tag="pstat", name=f"{pfx}q")
                for j in range(KO):
                    nc.tensor.matmul(sqB, ones128, xsqs[j],
                                     start=(j == 0), stop=(j == KO - 1))
                return sumB, sqB

            def emit_ln_apply(x, sumB, sqB, gcol, bcol, pfx):
                """Stat chain kept DVE-local (single ACT hop for the sqrt):
                cross-engine sem hops in this chain are the dominant HW
                stall (~30us/rep measured with the ACT-hopping version)."""
                muB = stat_pool.tile([128, NB], F32, tag="muB", name=f"{pfx}mu")
                nc.vector.tensor_scalar_mul(muB, sumB, inv_n)
                rB = stat_pool.tile([128, NB], F32, tag="rB", name=f"{pfx}r")
                nc.vector.tensor_scalar_mul(rB, sqB, inv_n)
                musqB = stat_pool.tile([128, NB], F32, tag="musqB",
                                       name=f"{pfx}musq")
                nc.vector.tensor_mul(musqB, muB, muB)
                nc.vector.tensor_sub(rB, rB, musqB)
                nc.scalar.activation(rB, rB,
                                     mybir.ActivationFunctionType.Sqrt,
                                     bias=eps_col)
                nc.vector.reciprocal(rB, rB)
                # x = ((x - muB) * rB) * g + beta   (g, beta per-partition)
                for j in range(KO):
                    nc.vector.tensor_sub(x[:, j, :], x[:, j, :], muB)
                    nc.vector.tensor_mul(x[:, j, :], x[:, j, :], rB)
                    nc.vector.tensor_scalar(
                        x[:, j, :], x[:, j, :],
                        consts[:, gcol + j: gcol + j + 1],
                        consts[:, bcol + j: bcol + j + 1],
                        op0=mybir.AluOpType.mult,
                        op1=mybir.AluOpType.add,
                    )

            tiles = [(rep, bt) for rep in range(reps) for bt in range(NBT)]
            hs_cur, hg_cur = prefetch_hs(*tiles[0], first=True)
            # late weight loads: W1/W2 aren't needed until the first MLP
            nc.sync.dma_start(out=w1_sb,
                              in_=w1.rearrange("(kc p) n -> p kc n", p=128))
            nc.sync.dma_start(out=w2_sb,
                              in_=w2.rearrange("(kc p) n -> p kc n", p=128))
            x1 = emit_vprime(*tiles[0], hs_cur, hg_cur)

            for i, (rep, bt) in enumerate(tiles):
                bsl = slice(0, NB) if timing else slice(bt * NB, (bt + 1) * NB)

                # prefetch tile t+1's activations a full tile ahead
                if i + 1 < len(tiles):
                    hs_nxt, hg_nxt = prefetch_hs(*tiles[i + 1])

                # ---- LN1 stat matmuls, then pipeline next tile's GEMM ----
                sumB, sqB = emit_ln_stats(x1, f"l1_{rep}_{bt}")
                if i + 1 < len(tiles):
                    x1_nxt = emit_vprime(*tiles[i + 1], hs_nxt, hg_nxt)
                emit_ln_apply(x1, sumB, sqB, _C_L1G, _C_L1B, f"l1_{rep}_{bt}")

                # ---- g = gelu(h_attn @ W1 + b1) ----
                g_sb = []
                for m in range(MH):
                    p1 = pmlp.tile([128, NB], F32, tag="pmlp",
                                   name=f"ps1{rep}_{bt}_{m}")
                    for k in range(KO):
                        nc.tensor.matmul(
                            p1,
                            w1_sb[:, k, m * 128:(m + 1) * 128],
                            x1[:, k, :],
                            start=(k == 0), stop=(k == KO - 1),
                        )
                    g = g_pool.tile([128, NB], MM_DT, tag="g",
                                    name=f"g{rep}_{bt}_{m}")
                    nc.scalar.activation(g, p1,
                                         mybir.ActivationFunctionType.Gelu,
                                         bias=consts[:, _C_B1 + m: _C_B1 + m + 1])
                    g_sb.append(g)

                # ---- x2 = g @ W2 + b2 + h_attn (two psum banks, m-pairs) ----
                x2 = out_pool.tile([128, KO, NB], MM_DT, tag="x2",
                                   name=f"x2_{rep}_{bt}")
                for m in range(KO):
                    p2 = pmlp.tile([128, NB], F32, tag="pmlp",
                                   name=f"ps2{rep}_{bt}_{m}")
                    for k in range(MH):
                        nc.tensor.matmul(
                            p2,
                            w2_sb[:, k, m * 128:(m + 1) * 128],
                            g_sb[k],
                            start=(k == 0), stop=(k == MH - 1),
                        )
                    # x2 = (ps2 + b2) + h_attn in one DVE op — avoids an
                    # ACT hop between the W2 MMs and the LN2 stat MMs
                    nc.vector.scalar_tensor_tensor(
                        x2[:, m, :], p2,
                        consts[:, _C_B2 + m: _C_B2 + m + 1],
                        x1[:, m, :],
                        op0=mybir.AluOpType.add,
                        op1=mybir.AluOpType.add,
                    )

                # ---- LN2 -> out (in place on x2) ----
                s2, q2 = emit_ln_stats(x2, f"l2_{rep}_{bt}")
                emit_ln_apply(x2, s2, q2, _C_L2G, _C_L2B, f"l2_{rep}_{bt}")

                nc.scalar.dma_start(out=outT_t[:, :, bsl], in_=x2)

                if timing and bt == NBT - 1:
                    nc.sync.dma_start(out=mark[0:1, 8 * rep: 8 * (rep + 1)],
                                      in_=mark_sb)
                if i + 1 < len(tiles):
                    x1 = x1_nxt

    if split_waits:
        _split_multi_waits(nc)
    return nc


def _bf16(a):
    import ml_dtypes
    return np.ascontiguousarray(np.asarray(a, np.float32)).astype(
        ml_dtypes.bfloat16)


def _chunk_cols(vec):
    """[n*128] -> [128, n] with column j = vec[j*128:(j+1)*128]."""
    return np.ascontiguousarray(vec.reshape(-1, 128).T.astype(np.float32))


def _make_consts(inputs):
    cst = np.concatenate(
        [
            _chunk_cols(np.asarray(inputs["b1"], np.float32)),
            _chunk_cols(np.asarray(inputs["b2"], np.float32)),
            _chunk_cols(np.asarray(inputs["ln1_g"], np.float32)),
            _chunk_cols(np.asarray(inputs["ln1_b"], np.float32)),
            _chunk_cols(np.asarray(inputs["ln2_g"], np.float32)),
            _chunk_cols(np.asarray(inputs["ln2_b"], np.float32)),
        ],
        axis=1,
    )
    assert cst.shape == (128, _C_N)
    return cst


def _shared_weights(inputs):
    Wv = np.asarray(inputs["Wv"], np.float32)
    Wo = np.asarray(inputs["Wo"], np.float32)
    wv_flat = Wv.transpose(1, 0, 2).reshape(S_DIM, HID)
    # fold the (dead-softmax) output projection into Wv: exact fp32 math
    wp = wv_flat @ Wo
    return {
        "wv": _bf16(wp),
        "w1": _bf16(np.asarray(inputs["W1"], np.float32)),
        "w2": _bf16(np.asarray(inputs["W2"], np.float32)),
        "cst": _make_consts(inputs),
    }


def _fold_bias(inputs):
    bv_flat = np.asarray(inputs["bv"], np.float32).reshape(HID)
    bo = np.asarray(inputs["bo"], np.float32)
    return bv_flat @ np.asarray(inputs["Wo"], np.float32) + bo


def _prepare_in_maps(inputs):
    h_g = np.asarray(inputs["h_g"], np.float32)
    h_s = np.asarray(inputs["h_s"], np.float32)
    bp = _fold_bias(inputs)
    shared = _shared_weights(inputs)
    in_maps = []
    for c in range(N_CORES):
        rows = slice(c * BL, (c + 1) * BL)
        in_maps.append({
            "hsT": _bf16(h_s[rows].T),
            # fold b' into the h_g residual: x1 = h_s@W' + (h_g + b')
            "hgT": _bf16(h_g[rows].T + bp[:, None]),
            **shared,
        })
    return in_maps


def _prepare_timing_in_maps(inputs):
    h_g = np.asarray(inputs["h_g"], np.float32)
    h_s = np.asarray(inputs["h_s"], np.float32)
    bp = _fold_bias(inputs)
    shared = _shared_weights(inputs)
    m = {
        "hsT": _bf16(h_s[:NB, :KSUB * 128].T),
        "hgT": _bf16(h_g[:NB].T + bp[:, None]),
        **shared,
    }
    m["wv"] = np.ascontiguousarray(m.pop("wv")[: KSUB * 128])
    return [dict(m) for _ in range(N_CORES)]


def _assemble(results):
    return np.ascontiguousarray(
        np.concatenate([np.asarray(r["outT"]).astype(np.float32).T
                        for r in results], axis=0))


def run(inputs, trace=False):
    nc = build_nc()
    in_maps = _prepare_in_maps(inputs)
    res = run_bass_kernel_spmd(nc, in_maps, list(range(N_CORES)), trace=trace)
    return _assemble(res.results), res


def kernel(**inputs):
    out, _ = run(inputs, trace=False)
    return out
